# revision 15
# baseline (speedup 1.0000x reference)
"""AdderNet layer (L1-distance "matmul" + bias scales + LayerNorm) on 8 TRN2 cores.

out[t, o] = LN_o(-sum_i |x[t,i]*bias_in[i] - w[i,o]| * bias_out[o])

Strategy (hardcoded for x:[2,2048,512], w:[512,512]):
  - Data-parallel over the 4096 flattened tokens: 512 tokens per core, weight
    replicated, no collectives. Host folds bias_in into x, sends -x^T f32.
  - Per token: DVE tensor_scalar (add+max, 4x bf16) + ScalarE Relu produce
    D_c = relu(w_c - x_t) tiles ([128 cin, 512 out], bf16). On alternating
    tokens DVE pre-pairs chunks 2+3 (tensor_tensor add, 2x bf16), trading
    spare DVE cycles for one fewer PE stream - on this toolchain the PE is
    hard-bound at one ingested column per cycle (no FWL / no weight-load
    overlap), so every dropped 512-column stream saves ~218ns of PE time.
  - TensorE reduces over cin as the moving operand (ones stationary):
    per token 3-4 matmuls accumulate into one [1,512] PSUM row; 4 tokens
    pack into one PSUM bank via tile_position column groups (32-row writes
    keep the bank fully initialized). ScalarE drains each 4-token bank
    (scale=2.0 fused) and a strided SBUF DMA assembles [token, out] tiles.
  - |z| = 2*relu(z) - z telescope: sum|w-x| = 2r - colw[o] + rowx[t]. rowx is
    constant along the LN axis so it cancels in LayerNorm; centered colw
    folds into one STT. Uniform bias_out/gamma/beta (the spec fills) fold
    into host-side constants (sign into gamma, eps/v^2 rescale).
"""

import functools

import numpy as np
import ml_dtypes

N_CORES = 8
CIN = 512
COUT = 512
NTOK = 4096  # 2*2048 flattened tokens
TOK_PER_CORE = NTOK // N_CORES  # 512
NCHUNK = CIN // 128  # 4 cin chunks
NGROUP = TOK_PER_CORE // 128  # 4 token groups
EPS = 1e-5

# ScalarE produces chunk 1's D tile on ACT_NUM of ACT_DEN tokens (measured
# HW ACT rate ~916ns/tile vs DVE ~194-303ns: a=0.82 is minimax-robust).
ACT_CHUNK = 1
ACT_NUM, ACT_DEN = 8, 9
# Pairing disabled: HW probes show saturated DVE/ACT run ~1.5x slower than
# the sim models (per-instruction issue overhead), so PE has slack and the
# TT pair trade (DVE +327ns to save PE 218ns) is a net loss on hardware.
PAIR_NUM, PAIR_DEN = 0, 2
N_WARM = 12  # dummy matmuls to warm the PE HAM clock gate during input DMAs


@functools.lru_cache(maxsize=2)
def _build_nc(fast: bool):
    import concourse.bacc as bacc
    import concourse.mybir as mybir
    from concourse.tile import TileContext

    f32 = mybir.dt.float32
    bf16 = mybir.dt.bfloat16
    Alu = mybir.AluOpType
    Act = mybir.ActivationFunctionType

    nc = bacc.Bacc(
        "TRN2",
        debug=False,
        enable_asserts=False,
        target_bir_lowering=False,
        num_devices=N_CORES,
    )

    xTn = nc.dram_tensor("xTn", [CIN, TOK_PER_CORE], f32, kind="ExternalInput").ap()
    w = nc.dram_tensor("w", [CIN, COUT], bf16, kind="ExternalInput").ap()
    colwc_b = nc.dram_tensor("colwc_b", [128, COUT], f32, kind="ExternalInput").ap()
    if fast:
        # cols: gscale=-sign(v)*gamma_u, beta_u, eps/v^2
        rowc = nc.dram_tensor("rowc", [128, 3], f32, kind="ExternalInput").ap()
    else:
        nbout_b = nc.dram_tensor(
            "nbout_b", [128, COUT], f32, kind="ExternalInput"
        ).ap()
        gamma_b = nc.dram_tensor(
            "gamma_b", [128, COUT], f32, kind="ExternalInput"
        ).ap()
        beta_b = nc.dram_tensor("beta_b", [128, COUT], f32, kind="ExternalInput").ap()
        rowx4 = nc.dram_tensor(
            "rowx4", [128, NGROUP], f32, kind="ExternalInput"
        ).ap()
    y = nc.dram_tensor("y", [TOK_PER_CORE, COUT], f32, kind="ExternalOutput").ap()

    with TileContext(nc) as tc:
        with (
            tc.tile_pool(name="const", bufs=1) as cpool,
            tc.tile_pool(name="dtiles", bufs=6) as dpool,
            tc.tile_pool(name="movps", bufs=2, space="PSUM") as mpool,
            tc.tile_pool(name="stage", bufs=2) as spool,
            tc.tile_pool(name="ln", bufs=2) as lpool,
        ):
            # ---- constants / weights ----
            ones32 = cpool.tile([128, 32], bf16, tag="ones32")
            nc.vector.memset(ones32, 1.0)
            ones_row = cpool.tile([128, COUT], bf16, tag="ones_row")
            nc.vector.memset(ones_row, 1.0)

            w_c = []
            for c in range(NCHUNK):
                wt = cpool.tile([128, COUT], bf16, tag=f"w{c}")
                nc.sync.dma_start(wt, w[c * 128 : (c + 1) * 128, :])
                w_c.append(wt)

            xn_c = []
            for c in range(NCHUNK):
                xr = cpool.tile([128, TOK_PER_CORE], f32, tag=f"xn{c}")
                nc.sync.dma_start(xr, xTn[c * 128 : (c + 1) * 128, :])
                xn_c.append(xr)

            cw_t = cpool.tile([128, COUT], f32, tag="cw")
            nc.sync.dma_start(cw_t, colwc_b[:, :])
            if fast:
                rc_t = cpool.tile([128, 3], f32, tag="rc")
                nc.sync.dma_start(rc_t, rowc[:, :])
            else:
                nb_t = cpool.tile([128, COUT], f32, tag="nb")
                nc.sync.dma_start(nb_t, nbout_b[:, :])
                ga_t = cpool.tile([128, COUT], f32, tag="ga")
                nc.sync.dma_start(ga_t, gamma_b[:, :])
                be_t = cpool.tile([128, COUT], f32, tag="be")
                nc.sync.dma_start(be_t, beta_b[:, :])
                rx_t = cpool.tile([128, NGROUP], f32, tag="rx")
                nc.sync.dma_start(rx_t, rowx4[:, :])

            # mov-part SBUF destination tiles (assembled by strided DMAs)
            smov = []
            for g in range(NGROUP):
                sg = spool.tile(
                    [128, COUT], f32, tag=f"smov{g}", name=f"smov{g}"
                )
                smov.append(sg)

            # Warm the PE HAM clock gate while the input DMAs run. Dummy
            # matmuls write into the first drain tile; every real token
            # group re-starts its slot with start=True, overwriting them.
            ps_first = mpool.tile([128, 4 * COUT], f32, tag="mv", name="psf")
            for k in range(N_WARM):
                nc.tensor.matmul(
                    ps_first[0:32, 0:COUT], ones32, ones_row,
                    start=True, stop=True, tile_position=(0, 0),
                )

            def emit_ln(g):
                msum = lpool.tile([128, 1], f32, tag="msum", name=f"msum{g}")
                if fast:
                    nc.vector.tensor_reduce(
                        msum, smov[g], mybir.AxisListType.X, Alu.add
                    )
                    mean = lpool.tile([128, 1], f32, tag="mean", name=f"mean{g}")
                    nc.vector.tensor_scalar(
                        mean, msum, 1.0 / COUT, None, Alu.mult
                    )
                    cent = lpool.tile([128, COUT], f32, tag="cent", name=f"cent{g}")
                    nc.vector.scalar_tensor_tensor(
                        cent, smov[g], mean[:, 0:1], cw_t,
                        Alu.subtract, Alu.subtract,
                    )
                    sq = lpool.tile([128, COUT], bf16, tag="sq", name=f"sq{g}")
                    vsum = lpool.tile([128, 1], f32, tag="vsum", name=f"vsum{g}")
                    nc.scalar.activation(sq, cent, Act.Square, accum_out=vsum)
                    veps = lpool.tile([128, 1], f32, tag="veps", name=f"veps{g}")
                    nc.vector.tensor_scalar(
                        veps, vsum, 1.0 / COUT, rc_t[:, 2:3], Alu.mult, Alu.add
                    )
                    sstd = lpool.tile([128, 1], f32, tag="sstd", name=f"sstd{g}")
                    nc.scalar.sqrt(sstd, veps)
                    rstd = lpool.tile([128, 1], f32, tag="rstd", name=f"rstd{g}")
                    nc.vector.reciprocal(rstd, sstd)
                    nrstd = lpool.tile([128, 1], f32, tag="nrstd", name=f"nrstd{g}")
                    nc.vector.tensor_scalar(
                        nrstd, rstd, rc_t[:, 0:1], None, Alu.mult
                    )
                    yt = lpool.tile([128, COUT], f32, tag="yt", name=f"yt{g}")
                    nc.vector.tensor_scalar(
                        yt, cent, nrstd[:, 0:1], rc_t[:, 1:2], Alu.mult, Alu.add
                    )
                    nc.sync.dma_start(y[g * 128 : (g + 1) * 128, :], yt)
                else:
                    s1 = lpool.tile([128, COUT], f32, tag="s1", name=f"s1{g}")
                    nc.vector.tensor_scalar(
                        s1, smov[g], rx_t[:, g : g + 1], None, Alu.add
                    )
                    z = lpool.tile([128, COUT], f32, tag="z", name=f"z{g}")
                    nc.vector.scalar_tensor_tensor(
                        z, s1, 0.0, cw_t, Alu.add, Alu.subtract
                    )
                    pre = lpool.tile([128, COUT], f32, tag="pre", name=f"pre{g}")
                    nc.vector.tensor_tensor(pre, z, nb_t, Alu.mult)
                    msum2 = lpool.tile([128, 1], f32, tag="msum2", name=f"ms2{g}")
                    nc.vector.tensor_reduce(
                        msum2, pre, mybir.AxisListType.X, Alu.add
                    )
                    mean = lpool.tile([128, 1], f32, tag="mean", name=f"mean{g}")
                    nc.vector.tensor_scalar(
                        mean, msum2, 1.0 / COUT, None, Alu.mult
                    )
                    cent = lpool.tile([128, COUT], f32, tag="cent", name=f"cent{g}")
                    nc.vector.tensor_scalar(
                        cent, pre, mean[:, 0:1], None, Alu.subtract
                    )
                    sq = lpool.tile([128, COUT], bf16, tag="sq", name=f"sq{g}")
                    vsum = lpool.tile([128, 1], f32, tag="vsum", name=f"vsum{g}")
                    nc.scalar.activation(sq, cent, Act.Square, accum_out=vsum)
                    veps = lpool.tile([128, 1], f32, tag="veps", name=f"veps{g}")
                    nc.vector.tensor_scalar(
                        veps, vsum, 1.0 / COUT, EPS, Alu.mult, Alu.add
                    )
                    sstd = lpool.tile([128, 1], f32, tag="sstd", name=f"sstd{g}")
                    nc.scalar.sqrt(sstd, veps)
                    rstd = lpool.tile([128, 1], f32, tag="rstd", name=f"rstd{g}")
                    nc.vector.reciprocal(rstd, sstd)
                    t1 = lpool.tile([128, COUT], f32, tag="t1", name=f"t1{g}")
                    nc.vector.scalar_tensor_tensor(
                        t1, cent, rstd[:, 0:1], ga_t, Alu.mult, Alu.mult
                    )
                    yt = lpool.tile([128, COUT], f32, tag="yt", name=f"yt{g}")
                    nc.vector.tensor_tensor(yt, t1, be_t, Alu.add)
                    nc.sync.dma_start(y[g * 128 : (g + 1) * 128, :], yt)

            # ---- main loop ----
            ps16 = None
            for t in range(TOK_PER_CORE):
                tt = t % 4           # PSUM column group
                blk = (t % 16) // 4  # free-dim quarter of the 4-bank tile
                if t % 16 == 0:
                    ps16 = (
                        ps_first
                        if t == 0
                        else mpool.tile([128, 4 * COUT], f32, tag="mv")
                    )
                d = [None] * NCHUNK
                for c in range(NCHUNK):
                    dt_ = dpool.tile([128, COUT], bf16, tag=f"d{c}")
                    if c == ACT_CHUNK and (t % ACT_DEN) < ACT_NUM:
                        nc.scalar.activation(
                            dt_, w_c[c], Act.Relu,
                            bias=xn_c[c][:, t : t + 1], scale=1.0,
                        )
                    else:
                        nc.vector.tensor_scalar(
                            dt_, w_c[c], xn_c[c][:, t : t + 1], 0.0,
                            Alu.add, Alu.max,
                        )
                    d[c] = dt_
                if (t % PAIR_DEN) < PAIR_NUM:
                    d23 = dpool.tile([128, COUT], bf16, tag="d23")
                    nc.vector.tensor_tensor(d23, d[2], d[3], Alu.add)
                    streams = [d[0], d[1], d23]
                else:
                    streams = [d[0], d[1], d[2], d[3]]
                last = len(streams) - 1
                for si, s in enumerate(streams):
                    nc.tensor.matmul(
                        ps16[32 * tt : 32 * tt + 32,
                             blk * COUT : (blk + 1) * COUT],
                        ones32, s,
                        start=(si == 0), stop=(si == last),
                        tile_position=(0, 32 * tt),
                    )
                if t % 16 == 15:
                    # drain 16 tokens in one ScalarE pass (amortizes the
                    # ~490ns per-instruction ACT overhead over 4x the data)
                    stg = dpool.tile([128, 4 * COUT], f32, tag="stg")
                    nc.scalar.activation(stg, ps16, Act.Copy, scale=2.0)
                    g, t0 = divmod(t - 15, 128)
                    for b4 in range(4):
                        nc.sync.dma_start(
                            smov[g][t0 + 4 * b4 : t0 + 4 * b4 + 4, :],
                            stg[0:128:32, b4 * COUT : (b4 + 1) * COUT],
                        )

            for g in range(NGROUP):
                emit_ln(g)

    nc.finalize()
    return nc


def _uniform(a):
    a = np.asarray(a, np.float32).reshape(-1)
    return bool(np.all(a == a[0]))


def _prep_inputs(x, weight, multi_bias_in, multi_bias_out, ln_gamma, ln_beta):
    x2 = np.asarray(x, np.float32).reshape(NTOK, CIN)
    xb = x2 * np.asarray(multi_bias_in, np.float32).reshape(1, CIN)
    w_bf = np.asarray(weight, np.float32).astype(ml_dtypes.bfloat16)
    bout = np.asarray(multi_bias_out, np.float32).reshape(COUT)
    gam = np.asarray(ln_gamma, np.float32).reshape(COUT)
    bet = np.asarray(ln_beta, np.float32).reshape(COUT)
    # colw from the bf16 weights actually used on device
    colw = w_bf.astype(np.float32).sum(axis=0)  # [COUT]

    fast = _uniform(bout) and _uniform(gam) and _uniform(bet) and bout[0] != 0.0
    if fast:
        v = float(bout[0])
        colwc = colw - colw.mean()
        extra = {
            "colwc_b": np.ascontiguousarray(
                np.broadcast_to(colwc.reshape(1, COUT), (128, COUT))
            ),
            "rowc": np.ascontiguousarray(
                np.stack(
                    [
                        np.full(128, -np.sign(v) * float(gam[0]), np.float32),
                        np.full(128, float(bet[0]), np.float32),
                        np.full(128, EPS / (v * v), np.float32),
                    ],
                    axis=1,
                )
            ),
        }
    else:
        extra = {
            "colwc_b": np.ascontiguousarray(
                np.broadcast_to(colw.reshape(1, COUT), (128, COUT))
            ),
            "nbout_b": np.ascontiguousarray(
                np.broadcast_to(-bout.reshape(1, COUT), (128, COUT))
            ),
            "gamma_b": np.ascontiguousarray(
                np.broadcast_to(gam.reshape(1, COUT), (128, COUT))
            ),
            "beta_b": np.ascontiguousarray(
                np.broadcast_to(bet.reshape(1, COUT), (128, COUT))
            ),
        }

    in_maps = []
    for k in range(N_CORES):
        shard = xb[k * TOK_PER_CORE : (k + 1) * TOK_PER_CORE, :]  # [tok, cin]
        xTn = np.ascontiguousarray(-shard.T)  # [cin, tok] f32, negated
        m = {"xTn": xTn, "w": w_bf, **extra}
        if not fast:
            rowx = shard.sum(axis=1)  # [tok] = sum_i x[t, i]
            m["rowx4"] = np.ascontiguousarray(rowx.reshape(NGROUP, 128).T)
        in_maps.append(m)
    return in_maps, fast


def _run(in_maps, fast, trace=False, trace_cores=None):
    from concourse import bass_utils

    nc = _build_nc(fast)
    return bass_utils.run_bass_kernel_spmd(
        nc,
        in_maps,
        core_ids=list(range(N_CORES)),
        trace=trace,
        trace_cores=trace_cores,
    )


def kernel(x, weight, multi_bias_in, multi_bias_out, ln_gamma, ln_beta):
    in_maps, fast = _prep_inputs(
        x, weight, multi_bias_in, multi_bias_out, ln_gamma, ln_beta
    )
    res = _run(in_maps, fast)
    out = np.concatenate([r["y"] for r in res.results], axis=0)
    return out.reshape(np.asarray(x).shape[:-1] + (COUT,)).astype(np.float32)
